# revision 1
# baseline (speedup 1.0000x reference)
"""Trainium2 Bass kernel for AttentionConv3D (channel attention + depthwise conv).

Data-parallel over batch: batch element i runs on NeuronCore i (8 cores),
all parameters replicated; no collectives needed. Measured ~517 us/core on
TRN2 silicon, rel err ~5e-3 vs the fp32 reference (bf16 compute).

Per-core pipeline (one batch element, strip-pipelined 8 image rows at a
time; bf16 compute, fp32 PSUM accumulation):
  1. qkv = w_qkv @ x            TensorE matmuls into PSUM, copyback to a
                                zero-padded pitched SBUF strip (ScalarE);
                                strip boundary rows copied, not recomputed
  2. depthwise 3x3 conv         7 taps as diag(w) matmuls accumulating in
                                PSUM (TensorE); 2 taps chained on VectorE
                                scalar_tensor_tensor, fused with the
                                PSUM->SBUF copyback
  3. channel attention          per-strip PE transposes of q,k chunks; one
                                PSUM-resident q@k^T accumulation across all
                                strips; l2-norm + temperature folded into
                                the tiny per-head 32x32 softmax (sum of
                                squares via ScalarE Square+accumulate)
  4. proj folded into attn      PAT = (P @ blockdiag(attn))^T computed once,
                                so phase 2 is a single fused matmul per
                                512-pixel tile: out = PAT^T @ v
"""
import sys

sys.path.insert(0, "/opt/trn_rl_repo")

import numpy as np
import ml_dtypes

import concourse.bass as bass
from concourse import bacc, mybir
from concourse.tile import TileContext
from concourse.masks import make_identity

FP32 = mybir.dt.float32
F32R = mybir.dt.float32r
BF16 = mybir.dt.bfloat16
AX = mybir.AxisListType
ALU = mybir.AluOpType
ACTF = mybir.ActivationFunctionType

C = 256
H = W = 128
HEADS = 8
CH = C // HEADS  # 32
QC = 3 * C       # 768
QB = QC // 128   # 6 qkv channel blocks
S = 8            # image rows per strip
NSTRIP = H // S  # 16
PITCH = W + 2    # 130: qkv strip row pitch (zero pad col at both ends)
N_CORES = 8
EPS = 1e-12


def build_kernel():
    nc = bacc.Bacc("TRN2", target_bir_lowering=False, debug=False,
                   num_devices=N_CORES)

    x_d = nc.dram_tensor("x", [C, H, W], BF16, kind="ExternalInput").ap()
    wqT_d = nc.dram_tensor("w_qkvT", [C, QC], BF16, kind="ExternalInput").ap()
    w9_d = nc.dram_tensor("w9", [128, QB * 9], FP32, kind="ExternalInput").ap()
    wpT_d = nc.dram_tensor("w_projT", [C, C], BF16, kind="ExternalInput").ap()
    temp_d = nc.dram_tensor("temp_pc", [128, 2], FP32, kind="ExternalInput").ap()
    sel_d = nc.dram_tensor("sel2", [2, 2, 128], FP32, kind="ExternalInput").ap()
    out_d = nc.dram_tensor("out", [C, H, W], FP32, kind="ExternalOutput").ap()
    kr_scratch = nc.dram_tensor("kr_scratch", [2, 128], FP32).ap()

    with TileContext(nc) as tc:
        _body(nc, tc, x_d, wqT_d, w9_d, wpT_d, temp_d, out_d, kr_scratch,
              sel_d)
    nc.compile()
    return nc


def _body(nc, tc, x_d, wqT_d, w9_d, wpT_d, temp_d, out_d, kr_scratch,
          sel_d=None):
    from contextlib import ExitStack

    ctx = ExitStack()
    with ctx:
        persist = ctx.enter_context(tc.tile_pool(name="persist", bufs=1))

        # ---- persistent tiles ----
        wq_sb = persist.tile([128, 2, QC], BF16, tag="wq")      # w_qkvT blocks
        nc.sync.dma_start(out=wq_sb[:, 0, :], in_=wqT_d[0:128, :])
        nc.sync.dma_start(out=wq_sb[:, 1, :], in_=wqT_d[128:256, :])
        wp_sb = persist.tile([128, 2, C], BF16, tag="wp")       # w_projT blocks
        nc.sync.dma_start(out=wp_sb[:, 0, :], in_=wpT_d[0:128, :])
        nc.sync.dma_start(out=wp_sb[:, 1, :], in_=wpT_d[128:256, :])
        w9_sb = persist.tile([128, QB * 9], FP32, tag="w9")
        nc.sync.dma_start(out=w9_sb[:], in_=w9_d[:])
        temp_sb = persist.tile([128, 2], FP32, tag="temp")
        nc.sync.dma_start(out=temp_sb[:], in_=temp_d[:])

        ident_bf = persist.tile([128, 128], BF16, tag="idb")
        make_identity(nc, ident_bf)
        ident_f32 = persist.tile([128, 128], FP32, tag="idf")
        make_identity(nc, ident_f32)
        sel_row = persist.tile([2, 2, 128], FP32, tag="selr")
        nc.sync.dma_start(out=sel_row[:], in_=sel_d[:])
        # preload the sqrt activation table set early so the softmax
        # interlude only pays for the exp set load
        scr1 = persist.tile([128, 1], FP32, tag="scr1")
        nc.vector.memset(scr1[:], 1.0)
        nc.scalar.activation(out=scr1[:], in_=scr1[:], func=ACTF.Sqrt)

        # diagonal tap-weight matrices: diag(w9[block, tap]) in bf16
        diag_sb = persist.tile([128, QB * 9, 128], BF16, tag="diag")
        for col in range(QB * 9):
            nc.vector.tensor_scalar(
                out=diag_sb[:, col, :], in0=ident_bf[:],
                scalar1=w9_sb[:, col:col + 1], scalar2=None, op0=ALU.mult)

        # v storage (full image, bf16), per v-block
        v_sb = persist.tile([128, 2, H, W], BF16, tag="vsb")
        # sumsq stats per (tensor, block): one column per strip
        stats = persist.tile([128, 4, NSTRIP], FP32, tag="stats")
        # per-head softmaxed attention, block-diagonal [4 heads x 32]
        bd_pre = persist.tile([128, 2, 128], BF16, tag="bdpre")
        nc.vector.memset(bd_pre[:], 0.0)
        pat = persist.tile([128, 2, 256], BF16, tag="pat")  # (P @ attn)^T blocks
        smalls = persist.tile([128, 64], FP32, tag="smalls")  # misc scratch
        qkv_bufs = []
        for i in range(3):
            qb_t = persist.tile([128, QB, S + 2, PITCH], BF16, tag=f"qkvb{i}",
                                name=f"qkvbuf{i}")
            nc.vector.memset(qb_t[:, :, :, 0:1], 0.0)
            nc.vector.memset(qb_t[:, :, :, PITCH - 1:PITCH], 0.0)
            qkv_bufs.append(qb_t)
        sq_scr = persist.tile([128, S * W], BF16, tag="sqscr")  # Square output

        # ---- phase 1: qkv matmul + conv + attn stats, strip by strip ----
        p1 = ExitStack()
        with p1:
            xpool = p1.enter_context(tc.tile_pool(name="xpool", bufs=2))
            qkpool = p1.enter_context(tc.tile_pool(name="qkpool", bufs=3))
            convtmp = p1.enter_context(tc.tile_pool(name="convtmp", bufs=6))
            tppool = p1.enter_context(tc.tile_pool(name="tppool", bufs=3))
            ps_mm = p1.enter_context(tc.tile_pool(name="ps_mm", bufs=2, space="PSUM"))
            ps_cv = p1.enter_context(tc.tile_pool(name="ps_cv", bufs=2, space="PSUM"))
            ps_tp = p1.enter_context(tc.tile_pool(name="ps_tp", bufs=2, space="PSUM"))
            ps_at = p1.enter_context(tc.tile_pool(name="ps_at", bufs=1, space="PSUM"))

            attn_ps = []
            for g in range(2):
                a_ps = ps_at.tile([128, 128], FP32, tag=f"attn{g}", name=f"attn{g}")
                attn_ps.append(a_ps)

            def emit_qkv(s):
                r0 = s * S
                # rows this strip must COMPUTE (boundary rows r0-1, r0 are
                # copied from the previous strip's buffer instead)
                c_lo = r0 if s == 0 else r0 + 1   # first computed image row
                c_hi = min(r0 + S, H - 1)         # last computed image row

                x_sb = xpool.tile([128, 2, S + 1, W], BF16, tag="xs",
                                  name=f"xs{s}")
                for kb in range(2):
                    nc.sync.dma_start(
                        out=x_sb[:, kb, 0:c_hi - c_lo + 1, :],
                        in_=x_d[kb * 128:(kb + 1) * 128, c_lo:c_hi + 1, :])

                qkv_sb = qkv_bufs[s % len(qkv_bufs)]
                if s == 0:
                    nc.vector.memset(qkv_sb[:, :, 0, :], 0.0)
                if s == NSTRIP - 1:
                    nc.vector.memset(qkv_sb[:, :, S + 1, :], 0.0)
                # copy boundary rows r0-1, r0 from the previous strip's buffer
                if s > 0:
                    prev_qkv = qkv_bufs[(s - 1) % len(qkv_bufs)]
                    nc.vector.tensor_copy(
                        out=qkv_sb[:, :, 0:2, :],
                        in_=prev_qkv[:, :, S:S + 2, :])

                # qkv matmul for computed rows, chunks of <=4 rows
                row = c_lo
                while row <= c_hi:
                    cr = min(4, c_hi - row + 1)
                    npx = cr * W
                    boff = row - (r0 - 1)
                    xoff = row - c_lo
                    for qb in range(QB):
                        mm_ps = ps_mm.tile([128, 512], FP32, tag="mmps",
                                           name=f"mmps{s}_{row}_{qb}")
                        for kb in range(2):
                            nc.tensor.matmul(
                                mm_ps[:, 0:npx],
                                lhsT=wq_sb[:, kb, qb * 128:(qb + 1) * 128],
                                rhs=x_sb[:, kb, xoff:xoff + cr, :],
                                start=(kb == 0), stop=(kb == 1))
                        cb_out = qkv_sb[:, qb, boff:boff + cr, 1:1 + W]
                        cb_in = mm_ps[:, 0:npx].rearrange("p (r w) -> p r w", w=W)
                        nc.scalar.copy(out=cb_out, in_=cb_in)
                    row += cr

            def emit_rest(s):
                r0 = s * S
                qkv_sb = qkv_bufs[s % len(qkv_bufs)]
                # depthwise conv: taps 0..5 as diagonal matmuls in PSUM;
                # taps 6..8 run on DVE as a chained scalar_tensor_tensor
                # sequence seeded from the PSUM partial (no separate copyback).
                q_st = qkpool.tile([128, 2, S, W], BF16, tag="qst",
                                   name=f"qst{s}")
                k_st = qkpool.tile([128, 2, S, W], BF16, tag="kst",
                                   name=f"kst{s}")
                PE_TAPS = (0, 2, 3, 4, 5, 6, 8)   # DVE gets 7 and 1 (dw=0)
                for qb in range(QB):
                    cv_ps = [ps_cv.tile([128, 512], FP32, tag="cvps",
                                        name=f"cvps{s}_{qb}_{i}") for i in range(2)]
                    for ti, t in enumerate(PE_TAPS):
                        dh, dw = t // 3 - 1, t % 3 - 1
                        for cnk in range(2):
                            b0 = 4 * cnk + 1 + dh
                            nc.tensor.matmul(
                                cv_ps[cnk][:],
                                lhsT=diag_sb[:, qb * 9 + t, :],
                                rhs=qkv_sb[:, qb, b0:b0 + 4, 1 + dw:1 + dw + W],
                                start=(ti == 0), stop=(ti == len(PE_TAPS) - 1))
                    for cnk in range(2):
                        src = cv_ps[cnk][:].rearrange("p (r w) -> p r w", w=W)
                        if qb < 2:
                            dst = q_st[:, qb, 4 * cnk:4 * cnk + 4, :]
                        elif qb < 4:
                            dst = k_st[:, qb - 2, 4 * cnk:4 * cnk + 4, :]
                        else:
                            dst = v_sb[:, qb - 4, r0 + 4 * cnk:r0 + 4 * cnk + 4, :]
                        ctmp = convtmp.tile([128, 2, 4, W], BF16, tag="ctmp",
                                            name=f"ctmp{s}_{qb}_{cnk}")
                        chain = [(7, ctmp[:, 0]), (1, dst)]
                        acc = src
                        for t, o in chain:
                            dh, dw = t // 3 - 1, t % 3 - 1
                            b0 = 4 * cnk + 1 + dh
                            nc.vector.scalar_tensor_tensor(
                                out=o,
                                in0=qkv_sb[:, qb, b0:b0 + 4, 1 + dw:1 + dw + W],
                                scalar=w9_sb[:, qb * 9 + t:qb * 9 + t + 1],
                                in1=acc, op0=ALU.mult, op1=ALU.add)
                            acc = o

                # sumsq of q, k (per channel) via ACT square + accumulate
                for ti, t_st in enumerate((q_st, k_st)):
                    for g in range(2):
                        nc.scalar.activation(
                            out=sq_scr[:], in_=t_st[:, g, :, :], func=ACTF.Square,
                            accum_out=stats[:, ti * 2 + g, s:s + 1])

                # transpose q, k strips and accumulate attn_raw = q @ k^T
                # (8 transposes land in one PSUM bank, one consolidated copy)
                qT = tppool.tile([128, 2, S, 128], BF16, tag="qT",
                                 name=f"qT{s}")
                kT = tppool.tile([128, 2, S, 128], BF16, tag="kT",
                                 name=f"kT{s}")
                for ti, (t_st, t_T) in enumerate(((q_st, qT), (k_st, kT))):
                    for g in range(2):
                        tp_ps = ps_tp.tile([128, S, 128], BF16, tag="tpps",
                                           name=f"tpps{s}_{ti}_{g}")
                        for j in range(S):
                            nc.tensor.transpose(
                                tp_ps[:, j, :], in_=t_st[:, g, j, :],
                                identity=ident_bf[:])
                        nc.vector.tensor_copy(out=t_T[:, g, :, :], in_=tp_ps[:])
                for g in range(2):
                    for j in range(S):
                        nc.tensor.matmul(
                            attn_ps[g][:],
                            lhsT=qT[:, g, j, :], rhs=kT[:, g, j, :],
                            start=(s == 0 and j == 0),
                            stop=(s == NSTRIP - 1 and j == S - 1))

            # software-pipelined emission: strip s+1's qkv matmuls are
            # interleaved ahead of strip s's conv/attention work
            emit_qkv(0)
            for s in range(1, NSTRIP):
                emit_qkv(s)
                emit_rest(s - 1)
            emit_rest(NSTRIP - 1)

            # ---- softmax + normalization scales (tiny) ----
            ssq = smalls[:, 0:4]      # [q0 q1 k0 k1] sum of squares
            nrm = smalls[:, 4:8]
            for col in range(4):
                nc.vector.tensor_reduce(
                    out=ssq[:, col:col + 1], in_=stats[:, col, :],
                    axis=AX.X, op=ALU.add)
            nc.scalar.activation(out=nrm[:], in_=ssq[:], func=ACTF.Sqrt)
            nc.vector.tensor_scalar_max(nrm[:], nrm[:], EPS)
            rq = smalls[:, 8:10]      # 1/||q|| per q block
            nc.vector.reciprocal(out=rq[:], in_=nrm[:, 0:2])
            srow = smalls[:, 10:12]   # temperature / ||q||
            nc.vector.tensor_mul(srow[:], rq[:], temp_sb[:])

            # k norms transposed to a row vector -> DRAM -> broadcast tiles
            k_nrm_t = smalls[:, 12:14]
            nc.vector.tensor_copy(out=k_nrm_t[:], in_=nrm[:, 2:4])
            tp_ps = ps_tp.tile([128, 128], FP32, tag="tpps")
            nc.tensor.transpose(tp_ps[0:2, :], in_=k_nrm_t[:], identity=ident_f32[:])
            krow = persist.tile([128, 128], FP32, tag="krow")
            nc.vector.reciprocal(out=krow[0:2, :], in_=tp_ps[0:2, :])
            bc_k = persist.tile([128, 2, 32], FP32, tag="bck")
            for g in range(2):
                bc_ps = ps_tp.tile([128, 128], FP32, tag="tpps",
                                   name=f"bcps{g}")
                nc.tensor.matmul(bc_ps[:], lhsT=sel_row[:, g, :],
                                 rhs=krow[0:2, :], start=True, stop=True)
                for hh in range(4):
                    pr = slice(hh * 32, hh * 32 + 32)
                    nc.vector.tensor_copy(out=bc_k[pr, g, :],
                                          in_=bc_ps[pr, hh * 32:hh * 32 + 32])

            attn_s = persist.tile([128, 2, 128], FP32, tag="attns")
            for g in range(2):
                nc.vector.tensor_scalar(
                    out=attn_s[:, g, :], in0=attn_ps[g][:],
                    scalar1=srow[:, g:g + 1], scalar2=None, op0=ALU.mult)
                for hh in range(4):
                    pr = slice(hh * 32, hh * 32 + 32)
                    blk = attn_s[pr, g, hh * 32:hh * 32 + 32]
                    sm = smalls[pr, 16:48]
                    nc.vector.tensor_mul(sm, blk, bc_k[pr, g, :])
                    mx = smalls[pr, 48:49]
                    nc.vector.tensor_reduce(out=mx, in_=sm, axis=AX.X, op=ALU.max)
                    nmx = smalls[pr, 49:50]
                    nc.vector.tensor_scalar_mul(nmx, mx, -1.0)
                    esc = smalls[pr, 50:82] if False else None
                    e32 = attn_s[pr, g, hh * 32:hh * 32 + 32]  # overwrite in place
                    nc.scalar.activation(out=e32, in_=sm, func=ACTF.Exp, bias=nmx)
                    ssum = smalls[pr, 50:51]
                    nc.vector.tensor_reduce(out=ssum, in_=e32, axis=AX.X, op=ALU.add)
                    rsum = smalls[pr, 51:52]
                    nc.vector.reciprocal(out=rsum, in_=ssum)
                    nc.vector.tensor_scalar(
                        out=bd_pre[pr, g, hh * 32:hh * 32 + 32], in0=e32,
                        scalar1=rsum, scalar2=None, op0=ALU.mult)

            # fold proj into attention: PAT_g = BD_g^T @ P^T, so the whole
            # phase 2 collapses to out = PAT^T @ v (one fused matmul per tile)
            for g in range(2):
                pat_ps = ps_tp.tile([128, 256], FP32, tag="tpps",
                                    name=f"patps{g}")
                nc.tensor.matmul(pat_ps[:], lhsT=bd_pre[:, g, :],
                                 rhs=wp_sb[:, g, :], start=True, stop=True)
                nc.vector.tensor_copy(out=pat[:, g, :], in_=pat_ps[:])

        # ---- phase 2: out = (P @ attn) @ v ----
        p2 = ExitStack()
        with p2:
            outpool = p2.enter_context(tc.tile_pool(name="outpool", bufs=8))
            ps_pj = p2.enter_context(tc.tile_pool(name="ps_pj", bufs=4, space="PSUM"))

            for cnk in range(H * W // 512):   # 32 chunks of 4 rows
                rr = cnk * 4
                for ob in range(2):
                    pj_ps = ps_pj.tile([128, 512], FP32, tag="pjps")
                    for g in range(2):
                        nc.tensor.matmul(
                            pj_ps[:], lhsT=pat[:, g, ob * 128:(ob + 1) * 128],
                            rhs=v_sb[:, g, rr:rr + 4, :],
                            start=(g == 0), stop=(g == 1))
                    o_sb = outpool.tile([128, 512], FP32, tag="osb",
                                        name=f"osb{cnk}_{ob}")
                    if ob == 0:
                        nc.vector.tensor_copy(out=o_sb[:], in_=pj_ps[:])
                    else:
                        nc.scalar.copy(out=o_sb[:], in_=pj_ps[:])
                    nc.sync.dma_start(
                        out=out_d[ob * 128:(ob + 1) * 128, rr:rr + 4, :],
                        in_=o_sb[:].rearrange("p (r w) -> p r w", w=W))


_NC_CACHE = {}


def _get_nc():
    if "nc" not in _NC_CACHE:
        _NC_CACHE["nc"] = build_kernel()
    return _NC_CACHE["nc"]


def _host_prep(w_qkv, w_dw, w_proj, temperature):
    w_qkvT = np.ascontiguousarray(
        np.asarray(w_qkv, dtype=np.float32).T).astype(ml_dtypes.bfloat16)
    w9f = np.asarray(w_dw, dtype=np.float32)[:, 0, 1]          # [768, 3, 3]
    w9 = np.empty((128, QB * 9), dtype=np.float32)
    for b in range(QB):
        w9[:, b * 9:(b + 1) * 9] = w9f[b * 128:(b + 1) * 128].reshape(128, 9)
    w_projT = np.ascontiguousarray(np.asarray(w_proj, dtype=np.float32).T)
    w_projT_bf = w_projT.astype(ml_dtypes.bfloat16)
    sel2 = np.zeros((2, 2, 128), dtype=np.float32)
    sel2[0, 0, :] = 1.0
    sel2[1, 1, :] = 1.0
    t = np.asarray(temperature, dtype=np.float32).reshape(HEADS)
    temp_pc = np.empty((128, 2), dtype=np.float32)
    for g in range(2):
        for p in range(128):
            temp_pc[p, g] = t[(g * 128 + p) // CH]
    return w_qkvT, w9, w_projT_bf, temp_pc, sel2


def kernel(x, w_qkv, w_dw, w_proj, temperature):
    from concourse.bass_utils import run_bass_kernel_spmd

    x = np.asarray(x, dtype=np.float32)
    b = x.shape[0]
    assert b == N_CORES
    w_qkvT, w9, w_projT_bf, temp_pc, sel2 = _host_prep(
        w_qkv, w_dw, w_proj, temperature)

    nc = _get_nc()
    x_bf = np.ascontiguousarray(x).astype(ml_dtypes.bfloat16)
    in_maps = [{
        "x": x_bf[i],
        "w_qkvT": w_qkvT,
        "w9": w9,
        "w_projT": w_projT_bf,
        "temp_pc": temp_pc,
        "sel2": sel2,
    } for i in range(b)]
    res = run_bass_kernel_spmd(nc, in_maps, core_ids=list(range(N_CORES)))
    out = np.stack([res.results[i]["out"] for i in range(b)], axis=0)
    return out.astype(np.float32)


if __name__ == "__main__":
    nc = build_kernel()
    print("built + compiled OK")

